# revision 8
# baseline (speedup 1.0000x reference)
"""Bahdanau additive attention on 8 TRN2 NeuronCores.

Sharding: data-parallel over batch B=64 -> 8 batches per core. Weights
(W1, W2, V, b1, b2) replicated on every core.

Per-core math (8 batches, S=1024, D=512, A=512):
  feat[a, s]   = sum_d W1[d, a] * enc[s, d]            (PE, f32r, W1 stationary)
  tanh_feat    = tanh(feat + dec_feat[a, b] + b1 + b2) (ACT, fused bias, PSUM src)
  e[s]         = sum_a V[a] * tanh_feat[a, s]          (PE, M=1 matvec)
  attn         = exp(e - max) * mask / sum(...)        (DVE/ACT, per 4-batch group)
  context[d]   = sum_s attn[s] * enc[s, d]             (PE, M=1 matvec, natural enc)

The first matmul needs enc with D on partitions (transposed); context needs the
natural layout. fp32 DMA-transpose doesn't exist on TRN2, so the host feeds
both layouts (input marshaling at shard time).

bv is dropped on purpose: e enters the outputs only through softmax, and
softmax(e + bv) == softmax(e) exactly (shift invariance), including after the
mask-renormalization (the shift cancels in numerator and denominator).

Engine APs must start at a 32-aligned partition, so single-row results (e rows,
context rows) are staged at partition 0 and moved with small DMAs (which have
no partition-alignment restriction).
"""

import numpy as np

import concourse.bass as bass
import concourse.mybir as mybir
import concourse.tile as tile
from concourse import bacc
from concourse.bass import ds, ts
from concourse.bass_utils import run_bass_kernel_spmd
from concourse.masks import make_identity

F32 = mybir.dt.float32
F32R = mybir.dt.float32r
AF = mybir.ActivationFunctionType
ALU = mybir.AluOpType
AX = mybir.AxisListType

B, S, D, A = 64, 1024, 512, 512
NCORES = 8
BPC = B // NCORES          # 8 batches per core
P = 128
DCH = D // P               # 4 contraction chunks
ACH = A // P               # 4 attn-unit chunks
STL = S // P               # 8 token tiles per batch
NGRP = S // 512            # 2 moving-operand groups of 512 tokens per batch

GRPB = 4                   # batches per softmax group
NSG = BPC // GRPB          # number of softmax groups
ENC_BUFS = 3               # natural-enc tiles concurrently resident
ENCT_BUFS = 2
TANH_BUFS = 2


def build_nc():
    nc = bacc.Bacc("TRN2", target_bir_lowering=False)

    encT_h = nc.dram_tensor("encT", [BPC, D, S], F32R, kind="ExternalInput")
    enc_h = nc.dram_tensor("enc", [BPC, S, D], F32R, kind="ExternalInput")
    dec_h = nc.dram_tensor("dec", [BPC, D], F32, kind="ExternalInput")
    mask_h = nc.dram_tensor("mask", [BPC, S], F32, kind="ExternalInput")
    w1_h = nc.dram_tensor("W1", [D, A], F32R, kind="ExternalInput")
    w2_h = nc.dram_tensor("W2", [D, A], F32R, kind="ExternalInput")
    b1_h = nc.dram_tensor("b1", [A], F32, kind="ExternalInput")
    b2_h = nc.dram_tensor("b2", [A], F32, kind="ExternalInput")
    v_h = nc.dram_tensor("V", [A, 1], F32R, kind="ExternalInput")

    ctx_h = nc.dram_tensor("context", [BPC, D], F32, kind="ExternalOutput")
    attn_h = nc.dram_tensor("attn", [BPC, S], F32, kind="ExternalOutput")

    with tile.TileContext(nc) as tc:
        with (
            tc.tile_pool(name="const", bufs=1) as cpool,
            tc.tile_pool(name="attn_sb", bufs=2) as apool,
            tc.tile_pool(name="stage_sb", bufs=1) as spool,
        ):
            # ---- constants ----
            w1_sb = cpool.tile([P, DCH, A], F32R, tag="w1")
            nc.sync.dma_start(w1_sb[:], w1_h[:].rearrange("(dc p) a -> p dc a", p=P))
            w2_sb = cpool.tile([P, DCH, A], F32R, tag="w2")
            nc.sync.dma_start(w2_sb[:], w2_h[:].rearrange("(dc p) a -> p dc a", p=P))
            v_sb = cpool.tile([P, ACH], F32R, tag="v")
            nc.sync.dma_start(v_sb[:], v_h[:].rearrange("(ac p) o -> p (ac o)", p=P))
            b1_sb = cpool.tile([P, ACH], F32, tag="b1")
            nc.sync.dma_start(b1_sb[:], b1_h[:].rearrange("(ac p) -> p ac", p=P))
            b2_sb = cpool.tile([P, ACH], F32, tag="b2")
            nc.sync.dma_start(b2_sb[:], b2_h[:].rearrange("(ac p) -> p ac", p=P))
            dec_sb = cpool.tile([P, D], F32, tag="dec")
            nc.vector.memset(dec_sb[:], 0.0)
            nc.sync.dma_start(dec_sb[:BPC, :], dec_h[:])
            ident = cpool.tile([P, P], F32, tag="ident")
            make_identity(nc, ident)

            b12_sb = cpool.tile([P, ACH], F32, tag="b12")
            nc.vector.tensor_add(b12_sb[:], b1_sb[:], b2_sb[:])

            # combined tanh bias per (a-chunk, batch): dec@W2 + b1 + b2
            bias_sb = cpool.tile([P, ACH * BPC], F32, tag="bias")

            # ---- prologue: dec_hidden^T, dec_feat, fused bias ----
            with (
                tc.tile_pool(name="prol_ps", bufs=1, space="PSUM") as prol_ps,
                tc.tile_pool(name="prol_sb", bufs=1) as prol_sb,
            ):
                decT_sb = prol_sb.tile([P, DCH * BPC], F32R, tag="decT")
                for dc in range(DCH):
                    pt = prol_ps.tile([P, P], F32, tag="ptr")
                    nc.tensor.transpose(
                        pt[:], dec_sb[:, ds(dc * P, P)], ident[:]
                    )
                    nc.scalar.copy(decT_sb[:, ds(dc * BPC, BPC)], pt[:, :BPC])
                for ac in range(ACH):
                    pdf = prol_ps.tile([P, BPC], F32, tag="pdf")
                    for dc in range(DCH):
                        nc.tensor.matmul(
                            pdf[:],
                            lhsT=(w2_sb[:, dc, ds(ac * P, P)]),
                            rhs=(decT_sb[:, ds(dc * BPC, BPC)]),
                            start=(dc == 0),
                            stop=(dc == DCH - 1),
                        )
                    nc.scalar.activation(
                        bias_sb[:, ds(ac * BPC, BPC)],
                        pdf[:],
                        AF.Identity,
                        bias=b12_sb[:, ac : ac + 1],
                        scale=1.0,
                    )

            with (
                tc.tile_pool(name="encT_p", bufs=ENCT_BUFS) as encT_pool,
                tc.tile_pool(name="enc_p", bufs=ENC_BUFS) as enc_pool,
                tc.tile_pool(name="tanh_p", bufs=TANH_BUFS) as tanh_pool,
                tc.tile_pool(name="mm_ps", bufs=1, space="PSUM") as mm_ps,
            ):
                enc_tiles = [None] * BPC
                estage = None

                for b in range(BPC):
                    grp = b // GRPB
                    bb = b % GRPB
                    if bb == 0:
                        # partition-0 staging row for this group's e values
                        estage = spool.tile([1, GRPB * S], F32, tag="estage")

                    encT_sb = encT_pool.tile([P, DCH, S], F32R, tag="encT")
                    nc.sync.dma_start(
                        encT_sb[:], encT_h[b].rearrange("(dc p) s -> p dc s", p=P)
                    )
                    enc_sb = enc_pool.tile([P, STL, D], F32R, tag="encN")
                    nc.sync.dma_start(
                        enc_sb[:], enc_h[b].rearrange("(st p) d -> p st d", p=P)
                    )
                    enc_tiles[b] = enc_sb

                    tanh_sb = tanh_pool.tile([P, ACH, S], F32R, tag="tanh")
                    for ac in range(ACH):
                        fps = mm_ps.tile([P, S], F32, tag="feat", bufs=2)
                        for g in range(NGRP):
                            for dc in range(DCH):
                                nc.tensor.matmul(
                                    fps[:, ds(g * 512, 512)],
                                    lhsT=(w1_sb[:, dc, ds(ac * P, P)]),
                                    rhs=(encT_sb[:, dc, ds(g * 512, 512)]),
                                    start=(dc == 0),
                                    stop=(dc == DCH - 1),
                                )
                        nc.scalar.activation(
                            tanh_sb[:, ac, :],
                            fps[:],
                            AF.Tanh,
                            bias=bias_sb[:, ds(ac * BPC + b, 1)],
                            scale=1.0,
                        )

                    for g in range(NGRP):
                        eps = mm_ps.tile([1, 512], F32, tag="e", bufs=2)
                        for ac in range(ACH):
                            nc.tensor.matmul(
                                eps[:],
                                lhsT=(v_sb[:, ac : ac + 1]),
                                rhs=(tanh_sb[:, ac, ds(g * 512, 512)]),
                                start=(ac == 0),
                                stop=(ac == ACH - 1),
                            )
                        nc.vector.tensor_copy(
                            estage[0:1, ds(bb * S + g * 512, 512)], eps[:]
                        )

                    if bb == GRPB - 1:
                        # gather e rows onto partitions 0..3 (DMA: no partition
                        # alignment restriction)
                        attn_g = apool.tile([P, S], F32, tag="attng")
                        nc.vector.memset(attn_g[:], 0.0)
                        for i in range(GRPB):
                            nc.sync.dma_start(
                                attn_g[i : i + 1, :], estage[0:1, ds(i * S, S)]
                            )
                        mask_g = apool.tile([GRPB, S], F32, tag="maskg")
                        nc.sync.dma_start(mask_g[:], mask_h[ds(grp * GRPB, GRPB), :])

                        # ---- softmax + mask renorm on rows 0..3 ----
                        rows = attn_g[:GRPB, :]
                        nmax = apool.tile([GRPB, 1], F32, tag="nmax")
                        nc.vector.reduce_max(nmax[:], rows, axis=AX.X, negate=True)
                        nc.scalar.activation(
                            rows, rows, AF.Exp, bias=nmax[:], scale=1.0
                        )
                        nc.vector.tensor_tensor(
                            rows, rows, mask_g[:], ALU.mult
                        )
                        rsum = apool.tile([GRPB, 1], F32, tag="rsum")
                        nc.vector.reduce_sum(rsum[:], rows, axis=AX.X)
                        rinv = apool.tile([GRPB, 1], F32, tag="rinv")
                        nc.vector.reciprocal(rinv[:], rsum[:])
                        nc.vector.tensor_scalar_mul(rows, rows, rinv[:])
                        nc.sync.dma_start(attn_h[ds(grp * GRPB, GRPB), :], rows)

                        # ---- attn^T columns for the context matvec ----
                        # attn_g rows 4..31 are zero, so K=32 transposes are exact
                        attnT = apool.tile([P, STL * GRPB], F32R, tag="attnT")
                        for st in range(STL):
                            atps = mm_ps.tile([P, P], F32, tag="atps", bufs=1)
                            nc.tensor.transpose(
                                atps[:], attn_g[:, ds(st * P, P)], ident[:]
                            )
                            nc.vector.tensor_copy(
                                attnT[:, ds(st * GRPB, GRPB)], atps[:, :GRPB]
                            )

                        # ---- context = attn @ enc (natural layout) ----
                        cstage = spool.tile([1, GRPB * D], F32, tag="cstage")
                        for i in range(GRPB):
                            bi = grp * GRPB + i
                            cps = mm_ps.tile([1, D], F32, tag="ctx", bufs=1)
                            for st in range(STL):
                                nc.tensor.matmul(
                                    cps[:],
                                    lhsT=(attnT[:, ds(st * GRPB + i, 1)]),
                                    rhs=(enc_tiles[bi][:, st, :]),
                                    start=(st == 0),
                                    stop=(st == STL - 1),
                                )
                            nc.vector.tensor_copy(
                                cstage[0:1, ds(i * D, D)], cps[:]
                            )
                            nc.sync.dma_start(
                                ctx_h[bi : bi + 1, :], cstage[0:1, ds(i * D, D)]
                            )

    nc.finalize()
    return nc


_NC = None


def _get_nc():
    global _NC
    if _NC is None:
        _NC = build_nc()
    return _NC


def _make_in_maps(dec_hidden, enc_output, enc_padding_mask, W1, b1, W2, b2, V):
    f32 = np.float32
    enc_output = np.ascontiguousarray(np.asarray(enc_output, dtype=f32))
    dec_hidden = np.ascontiguousarray(np.asarray(dec_hidden, dtype=f32))
    enc_padding_mask = np.ascontiguousarray(np.asarray(enc_padding_mask, dtype=f32))
    W1 = np.ascontiguousarray(np.asarray(W1, dtype=f32))
    W2 = np.ascontiguousarray(np.asarray(W2, dtype=f32))
    b1 = np.ascontiguousarray(np.asarray(b1, dtype=f32))
    b2 = np.ascontiguousarray(np.asarray(b2, dtype=f32))
    V = np.ascontiguousarray(np.asarray(V, dtype=f32))

    in_maps = []
    for c in range(NCORES):
        sl = slice(c * BPC, (c + 1) * BPC)
        enc_c = enc_output[sl]
        in_maps.append(
            {
                "encT": np.ascontiguousarray(enc_c.transpose(0, 2, 1)),
                "enc": np.ascontiguousarray(enc_c),
                "dec": dec_hidden[sl],
                "mask": enc_padding_mask[sl],
                "W1": W1,
                "W2": W2,
                "b1": b1,
                "b2": b2,
                "V": V,
            }
        )
    return in_maps


def _run(in_maps, **kwargs):
    nc = _get_nc()
    return run_bass_kernel_spmd(nc, in_maps, core_ids=list(range(NCORES)), **kwargs)


def kernel(dec_hidden, enc_output, enc_padding_mask, W1, b1, W2, b2, V, bv):
    # bv enters only through softmax; softmax is shift-invariant so it cancels
    # exactly (see module docstring).
    in_maps = _make_in_maps(
        dec_hidden, enc_output, enc_padding_mask, W1, b1, W2, b2, V
    )
    res = _run(in_maps)
    context = np.concatenate([r["context"] for r in res.results], axis=0)
    attn = np.concatenate([r["attn"] for r in res.results], axis=0)
    return context, attn
